# revision 17
# baseline (speedup 1.0000x reference)
"""Trainium2 Bass kernel for nn_Attention_16355235463288.

Additive attention:
    q  = ht_query @ Wq.T                      [B, D]
    e  = tanh(ctx_key + q[:, None, None, :])  [B, H, W, D]
    s  = einsum('bhwd,d->bhw', e, Wa[0]) + ba [B, H, W]
    s  = exp(s - max(s)) * mask ; s /= (sum_hw(s) + 1e-10)
    ct = einsum('bchw,bhw->bc', ctx_val, s)   [B, C]
    returns (ct, s)

Sharding: pure data parallel, B=32 over 8 cores (BL=4 per core). Params
replicated. No collectives. The global max-subtract in the reference is
dropped: softmax ratios are invariant to the subtracted constant except
through the +1e-10 term, where the relative effect is ~1e-11; |s| is
bounded by sum|Wa| ~ 20 so exp() cannot overflow in fp32.

Per-core dataflow (each stage streams ~33.5 MB from HBM):
  stage 0: qT[d, b] via PE matmul from host-pretransposed WqT/htT (fp32).
  stage 1 (ctx_key):
    HWDGE loads key tiles [128(hw), 512(d)] fp32 (contiguous 2KB runs)
    -> PE transpose 128x128 blocks into PSUM [128(d), hw]
    -> ACT tanh(psum + qT_bias) writes bf16 SBUF (fused q-add + cast)
    -> PE matmul with zero-padded-column Wa weights -> s in PSUM [4, 512]
    -> DVE copy to SBUF s[4, 4096].
  softmax (tiny, [4, 4096]): ACT exp(s+ba), DVE mask-mul / reduce /
    reciprocal / tensor_scalar normalize.
  stage 2 (ctx_val):
    SWDGE cast-load val tiles [128(c), 4096(hw)] f32->bf16
    -> PE "selector" matmul broadcasts s_norm across 128 partitions
    -> ACT copies PSUM -> bf16 SBUF
    -> DVE tensor_tensor_reduce (val * s_bcast, sum over hw) -> ct column.
"""

import sys
import numpy as np

for _p in ("/opt/trn_rl_repo", "/opt/pypackages"):
    if _p not in sys.path:
        sys.path.append(_p)

B, H, W, D, N, C = 32, 64, 64, 512, 512, 512
NCORES = 8
BL = B // NCORES          # 4 batches per core
HW = H * W                # 4096
G = 1024                  # hw positions per stage-1 group
NG = HW // G              # 4 groups
TPG = G // 128            # 8 hw-tiles per group

_CACHE = {}


def _build_nc(parts="all"):
    import concourse.bass as bass
    import concourse.mybir as mybir
    import concourse.tile as tile
    from concourse import bacc
    from concourse.masks import make_identity
    from contextlib import ExitStack

    f32 = mybir.dt.float32
    bf16 = mybir.dt.bfloat16
    AF = mybir.ActivationFunctionType
    OP = mybir.AluOpType

    nc = bacc.Bacc(None, target_bir_lowering=False, debug=False)

    key_d = nc.declare_dram_parameter("key", [BL, HW, D], f32, isOutput=False)
    val_d = nc.declare_dram_parameter("val", [BL, C, HW], f32, isOutput=False)
    mask_d = nc.declare_dram_parameter("mask", [BL, HW], f32, isOutput=False)
    htT_d = nc.declare_dram_parameter("htT4", [128, 4, BL], f32, isOutput=False)
    wqT_d = nc.declare_dram_parameter("WqT4", [128, 4, D], f32, isOutput=False)
    wap_d = nc.declare_dram_parameter("wa_pad", [128, BL, 4, BL], f32, isOutput=False)
    sel_d = nc.declare_dram_parameter("sel", [BL, BL, 128], f32, isOutput=False)
    ba_d = nc.declare_dram_parameter("ba4", [BL, 1], f32, isOutput=False)
    ct_d = nc.declare_dram_parameter("out_ct", [BL, C], f32, isOutput=True)
    s_d = nc.declare_dram_parameter("out_s", [BL, HW], f32, isOutput=True)

    with tile.TileContext(nc) as tc, ExitStack() as ctx:
        const = ctx.enter_context(tc.tile_pool(name="const", bufs=1))
        keyp = ctx.enter_context(tc.tile_pool(name="keyp", bufs=2))
        eTp = ctx.enter_context(tc.tile_pool(name="eTp", bufs=2))
        valp = ctx.enter_context(tc.tile_pool(name="valp", bufs=2))
        sbcp = ctx.enter_context(tc.tile_pool(name="sbcp", bufs=2))
        scrp = ctx.enter_context(tc.tile_pool(name="scrp", bufs=2))
        smallp = ctx.enter_context(tc.tile_pool(name="smallp", bufs=1))
        psT = ctx.enter_context(tc.tile_pool(name="psT", bufs=2, space="PSUM"))
        psS = ctx.enter_context(tc.tile_pool(name="psS", bufs=2, space="PSUM"))
        psB = ctx.enter_context(tc.tile_pool(name="psB", bufs=1, space="PSUM"))

        identity = const.tile([128, 128], f32)
        make_identity(nc, identity)
        htT = const.tile([128, 4, BL], f32)
        nc.sync.dma_start(htT[:], htT_d[:])
        wqT = const.tile([128, 4, D], f32)
        nc.sync.dma_start(wqT[:], wqT_d[:])
        wap = const.tile([128, BL, 4, BL], bf16)
        nc.gpsimd.dma_start(wap[:], wap_d[:])
        sel = const.tile([BL, BL, 128], bf16)
        nc.gpsimd.dma_start(sel[:], sel_d[:])
        ba4 = const.tile([BL, 1], f32)
        nc.sync.dma_start(ba4[:], ba_d[:])
        mask_sb = const.tile([BL, HW], bf16)
        nc.gpsimd.dma_start(mask_sb[:], mask_d[:])
        ct_sb = const.tile([128, BL * 4], f32)

        # stage 0: qT[d_local, (k, b)] = sum_n Wq[d, n] * ht[b, n]
        ps_q = psS.tile([128, 4 * BL], f32, tag="pss")
        for k in range(4):
            for nk in range(4):
                nc.tensor.matmul(
                    ps_q[:, k * BL:(k + 1) * BL],
                    wqT[:, nk, k * 128:(k + 1) * 128],
                    htT[:, nk, :],
                    start=(nk == 0),
                    stop=(nk == 3),
                )
        qT = const.tile([128, 4 * BL], f32)
        nc.vector.tensor_copy(qT[:], ps_q[:])

        s_sb = smallp.tile([BL, HW], f32, tag="s_sb")
        nc.gpsimd.memset(s_sb[:], 0.0)

        # stage 1
        for b in range(BL):
            for g in range(NG):
                key_sb = keyp.tile([128, TPG, D], f32, tag="key")
                nc.sync.dma_start(
                    key_sb[:],
                    key_d[b, g * G:(g + 1) * G, :].rearrange(
                        "(t p) d -> p t d", p=128
                    ),
                )
                if parts == "dma":
                    continue
                eT = eTp.tile([128, 4, G], bf16, tag="eT")
                for k in range(4):
                    psT_t = psT.tile([128, G], f32, tag="pst")
                    if parts == "s01not":
                        nc.vector.memset(psT_t[:], 0.25)
                    else:
                        for t in range(TPG):
                            nc.tensor.transpose(
                                psT_t[:, t * 128:(t + 1) * 128],
                                key_sb[:, t, k * 128:(k + 1) * 128],
                                identity,
                            )
                    nc.scalar.activation(
                        eT[:, k, :],
                        psT_t[:],
                        AF.Tanh,
                        bias=qT[:, (k * BL + b):(k * BL + b) + 1],
                    )
                for c in range(G // 512):
                    ps_s = psS.tile([BL, 512], f32, tag="pss")
                    for k in range(4):
                        nc.tensor.matmul(
                            ps_s[:],
                            wap[:, b, k, :],
                            eT[:, k, c * 512:(c + 1) * 512],
                            start=(k == 0),
                            stop=(k == 3),
                        )
                    off = g * G + c * 512
                    # rows != b of ps_s are zero (zero-padded Wa columns),
                    # so an in-place add deposits row b without clobbering
                    nc.vector.tensor_tensor(
                        s_sb[:, off:off + 512],
                        s_sb[:, off:off + 512],
                        ps_s[:],
                        OP.add,
                    )

        if parts == "s01":
            nc.sync.dma_start(s_d[:], s_sb[:])
            nc.gpsimd.memset(ct_sb[:], 0.0)

        if parts == "s01sm":
            nc.gpsimd.memset(ct_sb[:], 0.0)
        do_sm = parts != "s01"
        do_s2 = parts not in ("s01", "s01sm")
        # softmax over hw per batch (no max-subtract; see module docstring)
        use_ttr = "nottr" not in parts
        if do_sm:
            s_e = smallp.tile([BL, HW], f32, tag="s_e")
            nc.scalar.activation(s_e[:], s_sb[:], AF.Exp, bias=ba4[:])
            s_m = smallp.tile([BL, HW], f32, tag="s_m")
            denom = smallp.tile([BL, 1], f32, tag="denom")
            if use_ttr:
                nc.vector.tensor_tensor_reduce(
                    out=s_m[:],
                    in0=s_e[:],
                    in1=mask_sb[:],
                    scale=1.0,
                    scalar=1e-10,
                    op0=OP.mult,
                    op1=OP.add,
                    accum_out=denom[:],
                )
            else:
                nc.vector.tensor_tensor(s_m[:], s_e[:], mask_sb[:], OP.mult)
                nc.vector.tensor_reduce(
                    denom[:], s_m[:], mybir.AxisListType.X, OP.add
                )
                nc.vector.tensor_scalar_add(denom[:], denom[:], 1e-10)
            recip = smallp.tile([BL, 1], f32, tag="recip")
            nc.vector.reciprocal(recip[:], denom[:])
            s_nb = smallp.tile([BL, HW], bf16, tag="s_nb")
            nc.vector.tensor_scalar_mul(s_nb[:], s_m[:], recip[:])
            # reuse s_sb's slot for the fp32 normalized output copy
            nc.vector.tensor_scalar_mul(s_sb[:], s_m[:], recip[:])
            nc.sync.dma_start(s_d[:], s_sb[:])

        # stage 2
        for b in range(BL if do_s2 else 0):
            sbc = sbcp.tile([128, HW], bf16, tag="sbc")
            for h2 in range(HW // 1024):
                ps_b = psB.tile([128, 1024], f32, tag="psb")
                for j in range(2):
                    off = h2 * 1024 + j * 512
                    nc.tensor.matmul(
                        ps_b[:, j * 512:(j + 1) * 512],
                        sel[:, b, :],
                        s_nb[:, off:off + 512],
                        start=True,
                        stop=True,
                    )
                nc.scalar.activation(
                    sbc[:, h2 * 1024:(h2 + 1) * 1024], ps_b[:], AF.Copy
                )
            for cc in range(4):
                val_sb = valp.tile([128, HW], bf16, tag="val")
                nc.gpsimd.dma_start(
                    val_sb[:], val_d[b, cc * 128:(cc + 1) * 128, :]
                )
                prod = scrp.tile([128, HW], bf16, tag="prod")
                idx = b * 4 + cc
                if use_ttr:
                    nc.vector.tensor_tensor_reduce(
                        out=prod[:],
                        in0=val_sb[:],
                        in1=sbc[:],
                        scale=1.0,
                        scalar=0.0,
                        op0=OP.mult,
                        op1=OP.add,
                        accum_out=ct_sb[:, idx:idx + 1],
                    )
                else:
                    nc.vector.tensor_tensor(prod[:], val_sb[:], sbc[:], OP.mult)
                    nc.vector.tensor_reduce(
                        ct_sb[:, idx:idx + 1],
                        prod[:],
                        mybir.AxisListType.X,
                        OP.add,
                    )

        nc.sync.dma_start(
            ct_d.rearrange("b (c p) -> p b c", p=128),
            ct_sb.rearrange("p (b c) -> p b c", b=BL),
        )

    nc.compile()
    return nc


def run_parts(in_maps, parts):
    """Debug helper: run a partial build on HW, return per-core results."""
    from concourse.bass_utils import run_bass_kernel_spmd

    nc = _build_nc(parts=parts)
    return run_bass_kernel_spmd(nc, in_maps, core_ids=list(range(NCORES)))


def _prep_in_maps(ctx_val, ctx_key, ctx_mask, ht_query, Wq, Wa, ba):
    key = np.ascontiguousarray(ctx_key.reshape(B, HW, D), dtype=np.float32)
    val = np.ascontiguousarray(ctx_val.reshape(B, C, HW), dtype=np.float32)
    mask = np.ascontiguousarray(ctx_mask.reshape(B, HW), dtype=np.float32)
    ht = np.asarray(ht_query, dtype=np.float32)
    Wq = np.asarray(Wq, dtype=np.float32)
    Wa = np.asarray(Wa, dtype=np.float32)
    ba = np.asarray(ba, dtype=np.float32)

    # WqT4[n_local, nk, d] = Wq[d, nk*128 + n_local]
    WqT4 = np.ascontiguousarray(
        Wq.T.reshape(4, 128, D).transpose(1, 0, 2), dtype=np.float32
    )
    # wa_pad[d_local, b, k, col] = Wa[0, k*128+d_local] if col == b else 0
    waT = Wa[0].reshape(4, 128).T  # [d_local, k]
    wa_pad = np.zeros((128, BL, 4, BL), dtype=np.float32)
    for b in range(BL):
        wa_pad[:, b, :, b] = waT
    # sel[b, p, m] = 1 if p == b
    sel = np.zeros((BL, BL, 128), dtype=np.float32)
    for b in range(BL):
        sel[b, b, :] = 1.0
    ba4 = np.full((BL, 1), float(ba[0]), dtype=np.float32)

    in_maps = []
    for core in range(NCORES):
        sl = slice(core * BL, (core + 1) * BL)
        ht_sh = ht[sl]  # [BL, N]
        # htT4[n_local, nk, b] = ht_sh[b, nk*128 + n_local]
        htT4 = np.ascontiguousarray(
            ht_sh.T.reshape(4, 128, BL).transpose(1, 0, 2), dtype=np.float32
        )
        in_maps.append(
            {
                "key": key[sl],
                "val": val[sl],
                "mask": mask[sl],
                "htT4": htT4,
                "WqT4": WqT4,
                "wa_pad": wa_pad,
                "sel": sel,
                "ba4": ba4,
            }
        )
    return in_maps


def _install_profile_shim():
    """Provide antenv.axon_hooks + disable artifact upload so that
    run_bass_kernel_spmd(trace=True) can capture NTFF profiles in this
    container (the boot-time hook install is absent here)."""
    import types
    import ctypes
    import contextlib

    if "antenv.axon_hooks" not in sys.modules:
        mod = types.ModuleType("antenv.axon_hooks")
        holder = {"h": None}
        mod.set_axon_ntff_profile_hook = lambda h: holder.update(h=h)
        mod.get_axon_ntff_profile_hook = lambda: holder["h"]
        sys.modules["antenv.axon_hooks"] = mod
        import antenv

        antenv.axon_hooks = mod

    from antenv.axon_hooks import (
        get_axon_ntff_profile_hook,
        set_axon_ntff_profile_hook,
    )

    if get_axon_ntff_profile_hook() is None:
        lib = ctypes.CDLL("/opt/axon/libaxon_pjrt.so")
        if hasattr(lib, "axon_start_nrt_profile"):
            lib.axon_start_nrt_profile.argtypes = [
                ctypes.POINTER(ctypes.c_int64),
                ctypes.c_size_t,
            ]
            lib.axon_start_nrt_profile.restype = ctypes.c_int64
            lib.axon_stop_nrt_profile.argtypes = [ctypes.c_char_p]
            lib.axon_stop_nrt_profile.restype = ctypes.c_int64

            @contextlib.contextmanager
            def _hook(output_dir, device_ids):
                import jax

                jax.devices()
                if device_ids:
                    ids = (ctypes.c_int64 * len(device_ids))(*device_ids)
                    rc = lib.axon_start_nrt_profile(ids, len(device_ids))
                else:
                    rc = lib.axon_start_nrt_profile(None, 0)
                if rc != 0:
                    raise RuntimeError(f"axon_start_nrt_profile rc={rc}")
                try:
                    yield
                finally:
                    n = lib.axon_stop_nrt_profile(str(output_dir).encode())
                    print(f"profile: {n} file(s) written to {output_dir}")

            set_axon_ntff_profile_hook(_hook)

    from concourse import bass_utils as bu

    bu.upload_artifacts = lambda tmpdir: f"local:{tmpdir}"


def kernel(ctx_val, ctx_key, ctx_mask, ht_query, Wq, Wa, ba, _trace=False):
    from concourse.bass_utils import run_bass_kernel_spmd

    if _trace:
        _install_profile_shim()

    if "nc" not in _CACHE:
        # tensor_tensor_reduce is skipped: it faults the NRT runtime on
        # this fleet (bisected 2026-08-03); plain TT + reduce works.
        _CACHE["nc"] = _build_nc(parts="all_nottr")
    nc = _CACHE["nc"]

    in_maps = _prep_in_maps(ctx_val, ctx_key, ctx_mask, ht_query, Wq, Wa, ba)
    res = run_bass_kernel_spmd(
        nc, in_maps, core_ids=list(range(NCORES)), trace=_trace
    )
    if _trace:
        print(f"HW exec time: {res.exec_time_ns} ns")
        _CACHE["exec_time_ns"] = res.exec_time_ns
        _CACHE["results_obj"] = res

    ct = np.concatenate([r["out_ct"] for r in res.results], axis=0)
    s = np.concatenate([r["out_s"] for r in res.results], axis=0)
    return ct.astype(np.float32), s.reshape(B, H, W).astype(np.float32)


# revision 20
# speedup vs baseline: 1.0380x; 1.0380x over previous
"""Trainium2 Bass kernel for nn_Attention_16355235463288.

Additive attention:
    q  = ht_query @ Wq.T                      [B, D]
    e  = tanh(ctx_key + q[:, None, None, :])  [B, H, W, D]
    s  = einsum('bhwd,d->bhw', e, Wa[0]) + ba [B, H, W]
    s  = exp(s - max(s)) * mask ; s /= (sum_hw(s) + 1e-10)
    ct = einsum('bchw,bhw->bc', ctx_val, s)   [B, C]
    returns (ct, s)

Sharding: pure data parallel, B=32 over 8 cores (BL=4 per core). Params
replicated. No collectives. The global max-subtract in the reference is
dropped: softmax ratios are invariant to the subtracted constant except
through the +1e-10 term, where the relative effect is ~1e-11; |s| is
bounded by sum|Wa| ~ 20 so exp() cannot overflow in fp32. The mask is
folded in as a host-precomputed log(mask) initializer of the score
accumulator, so exp(s + logmask) = exp(s) * mask with no mask multiply.

Per-core dataflow (each stage streams ~33.5 MB from HBM), per batch b so
stage 2 of batch b overlaps stage 1 of batch b+1:
  stage 0: qT[d, b] via PE matmul from host-pretransposed WqT/htT (fp32).
  stage 1 (ctx_key):
    SWDGE cast-loads key tiles [128(hw), 512(d)] f32->bf16
    -> PE transpose 128x128 bf16 blocks into PSUM [128(d), hw]
    -> ACT tanh(psum + qT_bias) -> bf16 SBUF (fused q-add)
    -> PE matmul with zero-padded-column Wa weights -> scores [4, 512]
    -> DVE in-place add into s_sb[4, HW] (rows != b get += 0).
  softmax(b): ACT exp(s_sb + ba) -> u bf16 with accum_out giving the
    denominator for free; DVE reciprocal; tiny PE matmul broadcasts
    recip[b] over 128 partitions.
  stage 2 (ctx_val):
    PE selector matmul broadcasts u[b] over 128 partitions
    -> ACT Copy with scale=recip128 (fused normalize) -> ubc bf16
    -> SWDGE cast-load val tiles [128(c), 4096(hw)] f32->bf16
    -> DVE mul + reduce over hw -> ct column.
  tail: one fp32 exp + normalize for the s output.

tensor_tensor_reduce is avoided: it faults the NRT runtime on this
fleet (bisected 2026-08-03); plain tensor_tensor + tensor_reduce works.
"""

import sys
import numpy as np

for _p in ("/opt/trn_rl_repo", "/opt/pypackages"):
    if _p not in sys.path:
        sys.path.append(_p)

B, H, W, D, N, C = 32, 64, 64, 512, 512, 512
NCORES = 8
BL = B // NCORES          # 4 batches per core
HW = H * W                # 4096
G = 1024                  # hw positions per stage-1 group
NG = HW // G              # 4 groups
TPG = G // 128            # 8 hw-tiles per group

_CACHE = {}


def _build_nc():
    import concourse.bass as bass
    import concourse.mybir as mybir
    import concourse.tile as tile
    from concourse import bacc
    from concourse.masks import make_identity
    from contextlib import ExitStack

    f32 = mybir.dt.float32
    bf16 = mybir.dt.bfloat16
    AF = mybir.ActivationFunctionType
    OP = mybir.AluOpType

    nc = bacc.Bacc(None, target_bir_lowering=False, debug=False)

    key_d = nc.declare_dram_parameter("key", [BL, HW, D], f32, isOutput=False)
    val_d = nc.declare_dram_parameter("val", [BL, C, HW], f32, isOutput=False)
    lmask_d = nc.declare_dram_parameter("lmask", [BL, HW], f32, isOutput=False)
    htT_d = nc.declare_dram_parameter("htT4", [128, 4, BL], f32, isOutput=False)
    wqT_d = nc.declare_dram_parameter("WqT4", [128, 4, D], f32, isOutput=False)
    wap_d = nc.declare_dram_parameter("wa_pad", [128, BL, 4, BL], f32, isOutput=False)
    sel_d = nc.declare_dram_parameter("sel", [BL, BL, 128], f32, isOutput=False)
    ba_d = nc.declare_dram_parameter("ba4", [BL, 1], f32, isOutput=False)
    ct_d = nc.declare_dram_parameter("out_ct", [BL, C], f32, isOutput=True)
    s_d = nc.declare_dram_parameter("out_s", [BL, HW], f32, isOutput=True)

    with tile.TileContext(nc) as tc, ExitStack() as ctx:
        const = ctx.enter_context(tc.tile_pool(name="const", bufs=1))
        keyp = ctx.enter_context(tc.tile_pool(name="keyp", bufs=2))
        eTp = ctx.enter_context(tc.tile_pool(name="eTp", bufs=2))
        valp = ctx.enter_context(tc.tile_pool(name="valp", bufs=2))
        sbcp = ctx.enter_context(tc.tile_pool(name="sbcp", bufs=2))
        scrp = ctx.enter_context(tc.tile_pool(name="scrp", bufs=2))
        smallp = ctx.enter_context(tc.tile_pool(name="smallp", bufs=1))
        up = ctx.enter_context(tc.tile_pool(name="up", bufs=2))
        psT = ctx.enter_context(tc.tile_pool(name="psT", bufs=2, space="PSUM"))
        psS = ctx.enter_context(tc.tile_pool(name="psS", bufs=2, space="PSUM"))
        psB = ctx.enter_context(tc.tile_pool(name="psB", bufs=2, space="PSUM"))

        identity = const.tile([128, 128], bf16)
        make_identity(nc, identity)
        htT = const.tile([128, 4, BL], f32)
        nc.sync.dma_start(htT[:], htT_d[:])
        wqT = const.tile([128, 4, D], f32)
        nc.sync.dma_start(wqT[:], wqT_d[:])
        wap = const.tile([128, BL, 4, BL], bf16)
        nc.gpsimd.dma_start(wap[:], wap_d[:])
        sel = const.tile([BL, BL, 128], bf16)
        nc.gpsimd.dma_start(sel[:], sel_d[:])
        sel32 = const.tile([BL, BL, 128], f32)
        nc.sync.dma_start(sel32[:], sel_d[:])
        ba4 = const.tile([BL, 1], f32)
        nc.sync.dma_start(ba4[:], ba_d[:])
        ct_sb = const.tile([128, BL * 4], f32)

        # stage 0: qT[d_local, (k, b)] = sum_n Wq[d, n] * ht[b, n]
        ps_q = psS.tile([128, 4 * BL], f32, tag="pss")
        for k in range(4):
            for nk in range(4):
                nc.tensor.matmul(
                    ps_q[:, k * BL:(k + 1) * BL],
                    wqT[:, nk, k * 128:(k + 1) * 128],
                    htT[:, nk, :],
                    start=(nk == 0),
                    stop=(nk == 3),
                )
        qT = const.tile([128, 4 * BL], f32)
        nc.vector.tensor_copy(qT[:], ps_q[:])

        # score accumulator, initialized with log(mask) so the mask is
        # applied for free by the exp
        s_sb = smallp.tile([BL, HW], f32, tag="s_sb")
        nc.sync.dma_start(s_sb[:], lmask_d[:])
        denom = smallp.tile([BL, 1], f32, tag="denom")
        recip = smallp.tile([BL, 1], f32, tag="recip")

        for b in range(BL):
            # ---- stage 1 for batch b ----
            for g in range(NG):
                key_sb = keyp.tile([128, TPG, D], bf16, tag="key")
                nc.gpsimd.dma_start(
                    key_sb[:],
                    key_d[b, g * G:(g + 1) * G, :].rearrange(
                        "(t p) d -> p t d", p=128
                    ),
                )
                eT = eTp.tile([128, 4, G], bf16, tag="eT")
                for k in range(4):
                    psT_t = psT.tile([128, G], bf16, tag="pst")
                    for t in range(TPG):
                        nc.tensor.transpose(
                            psT_t[:, t * 128:(t + 1) * 128],
                            key_sb[:, t, k * 128:(k + 1) * 128],
                            identity,
                        )
                    nc.scalar.activation(
                        eT[:, k, :],
                        psT_t[:],
                        AF.Tanh,
                        bias=qT[:, (k * BL + b):(k * BL + b) + 1],
                    )
                for c in range(G // 512):
                    ps_s = psS.tile([BL, 512], f32, tag="pss")
                    for k in range(4):
                        nc.tensor.matmul(
                            ps_s[:],
                            wap[:, b, k, :],
                            eT[:, k, c * 512:(c + 1) * 512],
                            start=(k == 0),
                            stop=(k == 3),
                        )
                    off = g * G + c * 512
                    # rows != b of ps_s are zero (zero-padded Wa columns),
                    # so an in-place add deposits row b without clobbering
                    nc.vector.tensor_tensor(
                        s_sb[:, off:off + 512],
                        s_sb[:, off:off + 512],
                        ps_s[:],
                        OP.add,
                    )

            # ---- softmax pieces for batch b (rows != b hold junk that is
            # masked out by the selector matmuls downstream) ----
            u = up.tile([BL, HW], bf16, tag="u")
            nc.scalar.activation(
                u[:], s_sb[:], AF.Exp, bias=ba4[:], accum_out=denom[:]
            )
            nc.vector.reciprocal(recip[:], denom[:])
            ps_r = psB.tile([128, 1024], f32, tag="psb")
            nc.tensor.matmul(
                ps_r[:, 0:1], sel32[:, b, :], recip[:], start=True, stop=True
            )
            recip128 = up.tile([128, 1], f32, tag="recip128")
            nc.vector.tensor_copy(recip128[:], ps_r[:, 0:1])

            # ---- stage 2 for batch b ----
            sbc = sbcp.tile([128, HW], bf16, tag="sbc")
            for h2 in range(HW // 1024):
                ps_b = psB.tile([128, 1024], f32, tag="psb")
                for j in range(2):
                    off = h2 * 1024 + j * 512
                    nc.tensor.matmul(
                        ps_b[:, j * 512:(j + 1) * 512],
                        sel[:, b, :],
                        u[:, off:off + 512],
                        start=True,
                        stop=True,
                    )
                nc.scalar.activation(
                    sbc[:, h2 * 1024:(h2 + 1) * 1024],
                    ps_b[:],
                    AF.Copy,
                    scale=recip128[:],
                )
            for cc in range(4):
                val_sb = valp.tile([128, HW], bf16, tag="val")
                nc.gpsimd.dma_start(
                    val_sb[:], val_d[b, cc * 128:(cc + 1) * 128, :]
                )
                prod = scrp.tile([128, HW], bf16, tag="prod")
                idx = b * 4 + cc
                nc.vector.tensor_tensor(prod[:], val_sb[:], sbc[:], OP.mult)
                nc.vector.tensor_reduce(
                    ct_sb[:, idx:idx + 1],
                    prod[:],
                    mybir.AxisListType.X,
                    OP.add,
                )

        # ---- tail: fp32 normalized s output ----
        s_e = smallp.tile([BL, HW], f32, tag="s_e")
        nc.scalar.activation(s_e[:], s_sb[:], AF.Exp, bias=ba4[:])
        nc.vector.tensor_scalar_mul(s_sb[:], s_e[:], recip[:])
        nc.sync.dma_start(s_d[:], s_sb[:])

        nc.sync.dma_start(
            ct_d.rearrange("b (c p) -> p b c", p=128),
            ct_sb.rearrange("p (b c) -> p b c", b=BL),
        )

    nc.compile()
    return nc


def _prep_in_maps(ctx_val, ctx_key, ctx_mask, ht_query, Wq, Wa, ba):
    key = np.ascontiguousarray(ctx_key.reshape(B, HW, D), dtype=np.float32)
    val = np.ascontiguousarray(ctx_val.reshape(B, C, HW), dtype=np.float32)
    mask = np.asarray(ctx_mask, dtype=np.float32).reshape(B, HW)
    with np.errstate(divide="ignore"):
        lmask = np.where(mask > 0, np.log(np.maximum(mask, 1e-38)), -1e30)
    lmask = np.ascontiguousarray(lmask, dtype=np.float32)
    ht = np.asarray(ht_query, dtype=np.float32)
    Wq = np.asarray(Wq, dtype=np.float32)
    Wa = np.asarray(Wa, dtype=np.float32)
    ba = np.asarray(ba, dtype=np.float32)

    # WqT4[n_local, nk, d] = Wq[d, nk*128 + n_local]
    WqT4 = np.ascontiguousarray(
        Wq.T.reshape(4, 128, D).transpose(1, 0, 2), dtype=np.float32
    )
    # wa_pad[d_local, b, k, col] = Wa[0, k*128+d_local] if col == b else 0
    waT = Wa[0].reshape(4, 128).T  # [d_local, k]
    wa_pad = np.zeros((128, BL, 4, BL), dtype=np.float32)
    for b in range(BL):
        wa_pad[:, b, :, b] = waT
    # sel[b, p, m] = 1 if p == b
    sel = np.zeros((BL, BL, 128), dtype=np.float32)
    for b in range(BL):
        sel[b, b, :] = 1.0
    ba4 = np.full((BL, 1), float(ba[0]), dtype=np.float32)

    in_maps = []
    for core in range(NCORES):
        sl = slice(core * BL, (core + 1) * BL)
        ht_sh = ht[sl]  # [BL, N]
        # htT4[n_local, nk, b] = ht_sh[b, nk*128 + n_local]
        htT4 = np.ascontiguousarray(
            ht_sh.T.reshape(4, 128, BL).transpose(1, 0, 2), dtype=np.float32
        )
        in_maps.append(
            {
                "key": key[sl],
                "val": val[sl],
                "lmask": lmask[sl],
                "htT4": htT4,
                "WqT4": WqT4,
                "wa_pad": wa_pad,
                "sel": sel,
                "ba4": ba4,
            }
        )
    return in_maps


def _install_profile_shim():
    """Provide antenv.axon_hooks + disable artifact upload so that
    run_bass_kernel_spmd(trace=True) can capture NTFF profiles in this
    container (the boot-time hook install is absent here)."""
    import types
    import ctypes
    import contextlib

    if "antenv.axon_hooks" not in sys.modules:
        mod = types.ModuleType("antenv.axon_hooks")
        holder = {"h": None}
        mod.set_axon_ntff_profile_hook = lambda h: holder.update(h=h)
        mod.get_axon_ntff_profile_hook = lambda: holder["h"]
        sys.modules["antenv.axon_hooks"] = mod
        import antenv

        antenv.axon_hooks = mod

    from antenv.axon_hooks import (
        get_axon_ntff_profile_hook,
        set_axon_ntff_profile_hook,
    )

    if get_axon_ntff_profile_hook() is None:
        lib = ctypes.CDLL("/opt/axon/libaxon_pjrt.so")
        if hasattr(lib, "axon_start_nrt_profile"):
            lib.axon_start_nrt_profile.argtypes = [
                ctypes.POINTER(ctypes.c_int64),
                ctypes.c_size_t,
            ]
            lib.axon_start_nrt_profile.restype = ctypes.c_int64
            lib.axon_stop_nrt_profile.argtypes = [ctypes.c_char_p]
            lib.axon_stop_nrt_profile.restype = ctypes.c_int64

            @contextlib.contextmanager
            def _hook(output_dir, device_ids):
                import jax

                jax.devices()
                if device_ids:
                    ids = (ctypes.c_int64 * len(device_ids))(*device_ids)
                    rc = lib.axon_start_nrt_profile(ids, len(device_ids))
                else:
                    rc = lib.axon_start_nrt_profile(None, 0)
                if rc != 0:
                    raise RuntimeError(f"axon_start_nrt_profile rc={rc}")
                try:
                    yield
                finally:
                    n = lib.axon_stop_nrt_profile(str(output_dir).encode())
                    print(f"profile: {n} file(s) written to {output_dir}")

            set_axon_ntff_profile_hook(_hook)

    from concourse import bass_utils as bu

    bu.upload_artifacts = lambda tmpdir: f"local:{tmpdir}"


def kernel(ctx_val, ctx_key, ctx_mask, ht_query, Wq, Wa, ba, _trace=False):
    from concourse.bass_utils import run_bass_kernel_spmd

    if _trace:
        _install_profile_shim()

    if "nc" not in _CACHE:
        _CACHE["nc"] = _build_nc()
    nc = _CACHE["nc"]

    in_maps = _prep_in_maps(ctx_val, ctx_key, ctx_mask, ht_query, Wq, Wa, ba)
    res = run_bass_kernel_spmd(
        nc, in_maps, core_ids=list(range(NCORES)), trace=_trace
    )
    if _trace:
        print(f"HW exec time: {res.exec_time_ns} ns")
        _CACHE["exec_time_ns"] = res.exec_time_ns
        _CACHE["results_obj"] = res

    ct = np.concatenate([r["out_ct"] for r in res.results], axis=0)
    s = np.concatenate([r["out_s"] for r in res.results], axis=0)
    return ct.astype(np.float32), s.reshape(B, H, W).astype(np.float32)


# revision 23
# speedup vs baseline: 1.1501x; 1.1080x over previous
"""Trainium2 Bass kernel for nn_Attention_16355235463288.

Additive attention:
    q  = ht_query @ Wq.T                      [B, D]
    e  = tanh(ctx_key + q[:, None, None, :])  [B, H, W, D]
    s  = einsum('bhwd,d->bhw', e, Wa[0]) + ba [B, H, W]
    s  = exp(s - max(s)) * mask ; s /= (sum_hw(s) + 1e-10)
    ct = einsum('bchw,bhw->bc', ctx_val, s)   [B, C]
    returns (ct, s)

Sharding: pure data parallel, B=32 over 8 cores (BL=4 per core). Params
replicated. No collectives. The global max-subtract in the reference is
dropped: softmax ratios are invariant to the subtracted constant except
through the +1e-10 term, where the relative effect is ~1e-11; |s| is
bounded by sum|Wa| ~ 20 so exp() cannot overflow in fp32. The mask is
folded in as a host-precomputed log(mask) initializer of the score
accumulator, so exp(s + logmask) = exp(s) * mask with no mask multiply.

Per-core dataflow (each stage streams ~33.5 MB from HBM), per batch b so
stage 2 of batch b overlaps stage 1 of batch b+1:
  stage 0: qT[d, b] via PE matmul from host-pretransposed WqT/htT (fp32).
  stage 1 (ctx_key):
    SWDGE cast-loads key tiles [128(hw), 512(d)] f32->bf16
    -> PE transpose 128x128 bf16 blocks into PSUM [128(d), hw]
    -> ACT tanh(psum + qT_bias) -> bf16 SBUF (fused q-add)
    -> PE matmul with zero-padded-column Wa weights -> scores [4, 512]
    -> DVE in-place add into s_sb[4, HW] (rows != b get += 0).
  softmax(b): ACT exp(s_sb + ba) -> u bf16 with accum_out giving the
    denominator for free; DVE reciprocal; tiny PE matmul broadcasts
    recip[b] over 128 partitions.
  stage 2 (ctx_val):
    PE selector matmul broadcasts u[b] over 128 partitions
    -> ACT Copy with scale=recip128 (fused normalize) -> ubc bf16
    -> SWDGE cast-load val tiles [128(c), 4096(hw)] f32->bf16
    -> DVE mul + reduce over hw -> ct column.
  tail: one fp32 exp + normalize for the s output.

tensor_tensor_reduce is avoided: it faults the NRT runtime on this
fleet (bisected 2026-08-03); plain tensor_tensor + tensor_reduce works.
"""

import sys
import numpy as np

for _p in ("/opt/trn_rl_repo", "/opt/pypackages"):
    if _p not in sys.path:
        sys.path.append(_p)

B, H, W, D, N, C = 32, 64, 64, 512, 512, 512
NCORES = 8
BL = B // NCORES          # 4 batches per core
HW = H * W                # 4096
G = 1024                  # hw positions per stage-1 group
NG = HW // G              # 4 groups
TPG = G // 128            # 8 hw-tiles per group

_CACHE = {}


def _build_nc():
    import concourse.bass as bass
    import concourse.mybir as mybir
    import concourse.tile as tile
    from concourse import bacc
    from concourse.masks import make_identity
    from contextlib import ExitStack

    f32 = mybir.dt.float32
    bf16 = mybir.dt.bfloat16
    AF = mybir.ActivationFunctionType
    OP = mybir.AluOpType

    nc = bacc.Bacc(None, target_bir_lowering=False, debug=False)

    key_d = nc.declare_dram_parameter("key", [BL, HW, D], f32, isOutput=False)
    val_d = nc.declare_dram_parameter("val", [BL, C, HW], f32, isOutput=False)
    lmask_d = nc.declare_dram_parameter("lmask", [BL, HW], f32, isOutput=False)
    htT_d = nc.declare_dram_parameter("htT4", [128, 4, BL], f32, isOutput=False)
    wqT_d = nc.declare_dram_parameter("WqT4", [128, 4, D], f32, isOutput=False)
    wap_d = nc.declare_dram_parameter("wa_pad", [128, BL, 4, BL], f32, isOutput=False)
    sel_d = nc.declare_dram_parameter("sel", [BL, BL, 128], f32, isOutput=False)
    ba_d = nc.declare_dram_parameter("ba4", [BL, 1], f32, isOutput=False)
    dg_d = nc.declare_dram_parameter("diag4", [BL, BL * 4], f32, isOutput=False)
    ct_d = nc.declare_dram_parameter("out_ct", [BL, C], f32, isOutput=True)
    s_d = nc.declare_dram_parameter("out_s", [BL, HW], f32, isOutput=True)

    with tile.TileContext(nc) as tc, ExitStack() as ctx:
        const = ctx.enter_context(tc.tile_pool(name="const", bufs=1))
        keyp = ctx.enter_context(tc.tile_pool(name="keyp", bufs=3))
        eTp = ctx.enter_context(tc.tile_pool(name="eTp", bufs=2))
        valp = ctx.enter_context(tc.tile_pool(name="valp", bufs=4))
        sbcp = ctx.enter_context(tc.tile_pool(name="sbcp", bufs=2))
        scrp = ctx.enter_context(tc.tile_pool(name="scrp", bufs=2))
        smallp = ctx.enter_context(tc.tile_pool(name="smallp", bufs=1))
        up = ctx.enter_context(tc.tile_pool(name="up", bufs=2))
        psT = ctx.enter_context(tc.tile_pool(name="psT", bufs=2, space="PSUM"))
        psS = ctx.enter_context(tc.tile_pool(name="psS", bufs=2, space="PSUM"))
        psB = ctx.enter_context(tc.tile_pool(name="psB", bufs=2, space="PSUM"))

        identity = const.tile([128, 128], bf16)
        make_identity(nc, identity)
        htT = const.tile([128, 4, BL], f32)
        nc.sync.dma_start(htT[:], htT_d[:])
        wqT = const.tile([128, 4, D], f32)
        nc.sync.dma_start(wqT[:], wqT_d[:])
        wap = const.tile([128, BL, 4, BL], bf16)
        nc.gpsimd.dma_start(wap[:], wap_d[:])
        sel = const.tile([BL, BL, 128], bf16)
        nc.gpsimd.dma_start(sel[:], sel_d[:])
        sel32 = const.tile([BL, BL, 128], f32)
        nc.sync.dma_start(sel32[:], sel_d[:])
        ba4 = const.tile([BL, 1], f32)
        nc.sync.dma_start(ba4[:], ba_d[:])
        diag4 = const.tile([BL, BL * 4], f32)
        nc.sync.dma_start(diag4[:], dg_d[:])
        ones4 = const.tile([BL, 128], f32)
        nc.vector.memset(ones4[:], 1.0)
        ct_sb = const.tile([128, BL * 4], f32)

        # stage 0: qT[d_local, (k, b)] = sum_n Wq[d, n] * ht[b, n]
        ps_q = psS.tile([128, 4 * BL], f32, tag="pss")
        for k in range(4):
            for nk in range(4):
                nc.tensor.matmul(
                    ps_q[:, k * BL:(k + 1) * BL],
                    wqT[:, nk, k * 128:(k + 1) * 128],
                    htT[:, nk, :],
                    start=(nk == 0),
                    stop=(nk == 3),
                )
        qT = const.tile([128, 4 * BL], f32)
        nc.vector.tensor_copy(qT[:], ps_q[:])

        # score accumulator, initialized with log(mask) so the mask is
        # applied for free by the exp
        s_sb = smallp.tile([BL, HW], f32, tag="s_sb")
        nc.sync.dma_start(s_sb[:], lmask_d[:])
        denom = smallp.tile([BL, 1], f32, tag="denom")
        recip = smallp.tile([BL, 1], f32, tag="recip")

        for b in range(BL):
            val_tiles = {}
            # ---- stage 1 for batch b ----
            for g in range(NG):
                key_sb = keyp.tile([128, TPG, D], bf16, tag="key")
                nc.gpsimd.dma_start(
                    key_sb[:],
                    key_d[b, g * G:(g + 1) * G, :].rearrange(
                        "(t p) d -> p t d", p=128
                    ),
                )
                val_tiles[g] = valp.tile([128, HW], bf16, tag="val", name=f"val_{b}_{g}")
                nc.gpsimd.dma_start(
                    val_tiles[g][:], val_d[b, g * 128:(g + 1) * 128, :]
                )
                eT = eTp.tile([128, 4, G], bf16, tag="eT")
                for k in range(4):
                    psT_t = psT.tile([128, G], bf16, tag="pst")
                    for t in range(TPG):
                        nc.tensor.transpose(
                            psT_t[:, t * 128:(t + 1) * 128],
                            key_sb[:, t, k * 128:(k + 1) * 128],
                            identity,
                        )
                    nc.scalar.activation(
                        eT[:, k, :],
                        psT_t[:],
                        AF.Tanh,
                        bias=qT[:, (k * BL + b):(k * BL + b) + 1],
                    )
                for c in range(G // 512):
                    ps_s = psS.tile([BL, 512], f32, tag="pss")
                    for k in range(4):
                        nc.tensor.matmul(
                            ps_s[:],
                            wap[:, b, k, :],
                            eT[:, k, c * 512:(c + 1) * 512],
                            start=(k == 0),
                            stop=(k == 3),
                        )
                    off = g * G + c * 512
                    # rows != b of ps_s are zero (zero-padded Wa columns),
                    # so an in-place add deposits row b without clobbering
                    nc.vector.tensor_tensor(
                        s_sb[:, off:off + 512],
                        s_sb[:, off:off + 512],
                        ps_s[:],
                        OP.add,
                    )

            # ---- softmax pieces for batch b (rows != b hold junk that is
            # masked out by the selector matmuls downstream) ----
            u = up.tile([BL, HW], bf16, tag="u")
            nc.scalar.activation(
                u[:], s_sb[:], AF.Exp, bias=ba4[:], accum_out=denom[:]
            )
            nc.vector.reciprocal(recip[:], denom[:])

            # ---- stage 2 for batch b ----
            sbc = sbcp.tile([128, HW], bf16, tag="sbc")
            for h2 in range(HW // 1024):
                ps_b = psB.tile([128, 1024], f32, tag="psb")
                for j in range(2):
                    off = h2 * 1024 + j * 512
                    nc.tensor.matmul(
                        ps_b[:, j * 512:(j + 1) * 512],
                        sel[:, b, :],
                        u[:, off:off + 512],
                        start=True,
                        stop=True,
                    )
                nc.scalar.activation(
                    sbc[:, h2 * 1024:(h2 + 1) * 1024], ps_b[:], AF.Copy
                )
            for cc in range(4):
                val_sb = val_tiles[cc]
                prod = scrp.tile([128, HW], bf16, tag="prod")
                sum2 = scrp.tile([128, HW // 2], bf16, tag="sum2")
                idx = b * 4 + cc
                nc.vector.tensor_tensor(prod[:], val_sb[:], sbc[:], OP.mult)
                nc.vector.tensor_tensor(
                    sum2[:], prod[:, 0:HW // 2], prod[:, HW // 2:HW], OP.add
                )
                nc.vector.tensor_reduce(
                    ct_sb[:, idx:idx + 1],
                    sum2[:],
                    mybir.AxisListType.X,
                    OP.add,
                )

        # ---- tail: fp32 normalized s output ----
        s_e = smallp.tile([BL, HW], f32, tag="s_e")
        nc.scalar.activation(s_e[:], s_sb[:], AF.Exp, bias=ba4[:])
        nc.vector.tensor_scalar_mul(s_sb[:], s_e[:], recip[:])
        nc.sync.dma_start(s_d[:], s_sb[:])

        # normalize ct: recip_cols[p, (b,cc)] = recip[b]
        rdiag = smallp.tile([BL, BL * 4], f32, tag="rdiag")
        nc.vector.tensor_scalar_mul(rdiag[:], diag4[:], recip[:])
        ps_n = psB.tile([128, 1024], f32, tag="psb")
        nc.tensor.matmul(
            ps_n[:, 0:BL * 4], ones4[:], rdiag[:], start=True, stop=True
        )
        nc.vector.tensor_tensor(
            ct_sb[:], ct_sb[:], ps_n[:, 0:BL * 4], OP.mult
        )
        nc.sync.dma_start(
            ct_d.rearrange("b (c p) -> p b c", p=128),
            ct_sb.rearrange("p (b c) -> p b c", b=BL),
        )

    nc.compile()
    return nc


def _prep_in_maps(ctx_val, ctx_key, ctx_mask, ht_query, Wq, Wa, ba):
    key = np.ascontiguousarray(ctx_key.reshape(B, HW, D), dtype=np.float32)
    val = np.ascontiguousarray(ctx_val.reshape(B, C, HW), dtype=np.float32)
    mask = np.asarray(ctx_mask, dtype=np.float32).reshape(B, HW)
    with np.errstate(divide="ignore"):
        lmask = np.where(mask > 0, np.log(np.maximum(mask, 1e-38)), -1e30)
    lmask = np.ascontiguousarray(lmask, dtype=np.float32)
    ht = np.asarray(ht_query, dtype=np.float32)
    Wq = np.asarray(Wq, dtype=np.float32)
    Wa = np.asarray(Wa, dtype=np.float32)
    ba = np.asarray(ba, dtype=np.float32)

    # WqT4[n_local, nk, d] = Wq[d, nk*128 + n_local]
    WqT4 = np.ascontiguousarray(
        Wq.T.reshape(4, 128, D).transpose(1, 0, 2), dtype=np.float32
    )
    # wa_pad[d_local, b, k, col] = Wa[0, k*128+d_local] if col == b else 0
    waT = Wa[0].reshape(4, 128).T  # [d_local, k]
    wa_pad = np.zeros((128, BL, 4, BL), dtype=np.float32)
    for b in range(BL):
        wa_pad[:, b, :, b] = waT
    # sel[b, p, m] = 1 if p == b
    sel = np.zeros((BL, BL, 128), dtype=np.float32)
    for b in range(BL):
        sel[b, b, :] = 1.0
    ba4 = np.full((BL, 1), float(ba[0]), dtype=np.float32)
    # diag4[p, (b, cc)] = 1 if p == b (for the end-of-kernel ct normalize)
    diag4 = np.zeros((BL, BL * 4), dtype=np.float32)
    for b in range(BL):
        diag4[b, b * 4:(b + 1) * 4] = 1.0

    in_maps = []
    for core in range(NCORES):
        sl = slice(core * BL, (core + 1) * BL)
        ht_sh = ht[sl]  # [BL, N]
        # htT4[n_local, nk, b] = ht_sh[b, nk*128 + n_local]
        htT4 = np.ascontiguousarray(
            ht_sh.T.reshape(4, 128, BL).transpose(1, 0, 2), dtype=np.float32
        )
        in_maps.append(
            {
                "key": key[sl],
                "val": val[sl],
                "lmask": lmask[sl],
                "htT4": htT4,
                "WqT4": WqT4,
                "wa_pad": wa_pad,
                "sel": sel,
                "ba4": ba4,
                "diag4": diag4,
            }
        )
    return in_maps


def _install_profile_shim():
    """Provide antenv.axon_hooks + disable artifact upload so that
    run_bass_kernel_spmd(trace=True) can capture NTFF profiles in this
    container (the boot-time hook install is absent here)."""
    import types
    import ctypes
    import contextlib

    if "antenv.axon_hooks" not in sys.modules:
        mod = types.ModuleType("antenv.axon_hooks")
        holder = {"h": None}
        mod.set_axon_ntff_profile_hook = lambda h: holder.update(h=h)
        mod.get_axon_ntff_profile_hook = lambda: holder["h"]
        sys.modules["antenv.axon_hooks"] = mod
        import antenv

        antenv.axon_hooks = mod

    from antenv.axon_hooks import (
        get_axon_ntff_profile_hook,
        set_axon_ntff_profile_hook,
    )

    if get_axon_ntff_profile_hook() is None:
        lib = ctypes.CDLL("/opt/axon/libaxon_pjrt.so")
        if hasattr(lib, "axon_start_nrt_profile"):
            lib.axon_start_nrt_profile.argtypes = [
                ctypes.POINTER(ctypes.c_int64),
                ctypes.c_size_t,
            ]
            lib.axon_start_nrt_profile.restype = ctypes.c_int64
            lib.axon_stop_nrt_profile.argtypes = [ctypes.c_char_p]
            lib.axon_stop_nrt_profile.restype = ctypes.c_int64

            @contextlib.contextmanager
            def _hook(output_dir, device_ids):
                import jax

                jax.devices()
                if device_ids:
                    ids = (ctypes.c_int64 * len(device_ids))(*device_ids)
                    rc = lib.axon_start_nrt_profile(ids, len(device_ids))
                else:
                    rc = lib.axon_start_nrt_profile(None, 0)
                if rc != 0:
                    raise RuntimeError(f"axon_start_nrt_profile rc={rc}")
                try:
                    yield
                finally:
                    n = lib.axon_stop_nrt_profile(str(output_dir).encode())
                    print(f"profile: {n} file(s) written to {output_dir}")

            set_axon_ntff_profile_hook(_hook)

    from concourse import bass_utils as bu

    bu.upload_artifacts = lambda tmpdir: f"local:{tmpdir}"


def kernel(ctx_val, ctx_key, ctx_mask, ht_query, Wq, Wa, ba, _trace=False):
    from concourse.bass_utils import run_bass_kernel_spmd

    if _trace:
        _install_profile_shim()

    if "nc" not in _CACHE:
        _CACHE["nc"] = _build_nc()
    nc = _CACHE["nc"]

    in_maps = _prep_in_maps(ctx_val, ctx_key, ctx_mask, ht_query, Wq, Wa, ba)
    res = run_bass_kernel_spmd(
        nc, in_maps, core_ids=list(range(NCORES)), trace=_trace
    )
    if _trace:
        print(f"HW exec time: {res.exec_time_ns} ns")
        _CACHE["exec_time_ns"] = res.exec_time_ns
        _CACHE["results_obj"] = res

    ct = np.concatenate([r["out_ct"] for r in res.results], axis=0)
    s = np.concatenate([r["out_s"] for r in res.results], axis=0)
    return ct.astype(np.float32), s.reshape(B, H, W).astype(np.float32)
